# revision 13
# baseline (speedup 1.0000x reference)
"""TRN2 Bass kernel for 3-layer GAT + BN + MLP classifier (GATSBMs).

Self-contained: hardcodes all shapes. Accepts FULL inputs, returns FULL output.

Sharding: nodes sorted by in-degree desc -> rank r; core=r%8, slot j=r//8,
block b=j//128, lane p=j%128 (49 blocks of 128 lanes per core, NPAD=50176).
Each core owns its nodes' incoming edges. Per layer: each core computes
feat/el/er for its nodes (PE matmul vs W_ext=[W|W@albd|W@arbd]) in fp16,
AllGather of [feat|el] fp16 rows into a per-core 50176x132 table, then per
dst-block one indirect-DMA gather per edge-slot column into a channel-major
[128, 132, Cb] fp16 tile, edge softmax (normalized attention computed BEFORE
the message multiply so fp16 never overflows), weighted aggregation via one
fp16 multiply + one packed free-axis reduction on DVE, BN stats via PE
ones-matmuls + AllReduce.  Non-indirect DMA traffic is issued from the sync
(HWDGE) queue to keep the Pool/SWDGE engine free for the per-edge gathers,
which are the serial bottleneck.
"""
import numpy as np

N = 50000
E = 850000
HD = 128
H = 4
D = 32
NCORES = 8
BLK = 128
NB = 49
NPC = NB * BLK          # 6272
NPAD = NCORES * NPC     # 50176
NEG = 0.2
BN_EPS = 1e-5
EL_DUMMY = -100.0
F_EXT = 132             # feat(128) + el(4)

_CACHE = {}


def _build_plan(src, dst):
    """Vectorized host preprocessing of the graph structure."""
    src = np.asarray(src, dtype=np.int64)
    dst = np.asarray(dst, dtype=np.int64)
    deg = np.bincount(dst, minlength=N)
    order = np.argsort(-deg, kind="stable")
    rank_of = np.empty(N, dtype=np.int64)
    rank_of[order] = np.arange(N)

    deg_of_rank = np.zeros(NPAD, dtype=np.int64)
    deg_of_rank[:N] = deg[order]
    C = deg_of_rank[:1024 * NB].reshape(NB, 1024).max(axis=1)
    C = np.maximum(C, 1).astype(np.int64)
    off = np.zeros(NB, dtype=np.int64)
    np.cumsum(C[:-1], out=off[1:])
    CTOT = int(C.sum())

    DUMMY_ROW = NPC * ((NPAD - 1) % NCORES) + (NPAD - 1) // NCORES

    # per-edge placement
    es = np.argsort(dst, kind="stable")
    s_sorted = src[es]
    d_sorted = dst[es]
    starts = np.zeros(N + 1, dtype=np.int64)
    np.cumsum(deg, out=starts[1:])
    slot = np.arange(E, dtype=np.int64) - starts[d_sorted]
    dr = rank_of[d_sorted]
    k_e = dr % NCORES
    j_e = dr // NCORES
    b_e = j_e // BLK
    p_e = j_e % BLK
    srk = rank_of[s_sorted]
    row_e = NPC * (srk % NCORES) + srk // NCORES   # table row of the source node

    edge_idx = np.full((NCORES, BLK, CTOT), DUMMY_ROW, dtype=np.int32)
    flat_pos = k_e * (BLK * CTOT) + p_e * CTOT + off[b_e] + slot
    edge_idx.reshape(-1)[flat_pos] = row_e.astype(np.int32)

    # node-of-(k,b,p); -1 for dummy ranks
    nodes_kj = np.full((NCORES, NPC), -1, dtype=np.int64)
    r_all = 8 * np.tile(np.arange(NPC), (NCORES, 1)) + np.arange(NCORES)[:, None]
    m = r_all < N
    nodes_kj[m] = order[r_all[m]]
    return dict(C=C, off=off, CTOT=CTOT, edge_idx=edge_idx, nodes_kj=nodes_kj)


def _build_program(C, off, CTOT):
    import concourse.bass as bass
    import concourse.tile as tile
    from concourse import bacc, mybir
    from concourse.masks import make_identity

    f32 = mybir.dt.float32
    f16 = mybir.dt.float16
    i32 = mybir.dt.int32
    AF = mybir.ActivationFunctionType
    OP = mybir.AluOpType
    RG = [list(range(NCORES))]

    nc = bacc.Bacc(None, target_bir_lowering=False)

    # ---- external I/O ----
    emb_idx = nc.declare_dram_parameter("emb_idx", [BLK, NB], i32, isOutput=False)
    edge_idx = nc.declare_dram_parameter("edge_idx", [BLK, CTOT], i32, isOutput=False)
    snorm = nc.declare_dram_parameter("snorm", [BLK, NB], f32, isOutput=False)
    embed = nc.declare_dram_parameter("embed", [200, HD], f16, isOutput=False)
    Wexts = [nc.declare_dram_parameter(f"Wext{i}", [HD, HD + 8], f16, isOutput=False)
             for i in range(3)]
    gammas = [nc.declare_dram_parameter(f"gamma{i}", [HD, 1], f32, isOutput=False)
              for i in range(3)]
    betas = [nc.declare_dram_parameter(f"beta{i}", [HD, 1], f32, isOutput=False)
             for i in range(3)]
    cls1_w = nc.declare_dram_parameter("cls1_w", [HD, 64], f16, isOutput=False)
    cls1_b = nc.declare_dram_parameter("cls1_b", [64, 1], f32, isOutput=False)
    cls2_wb = nc.declare_dram_parameter("cls2_wb", [65, 2], f16, isOutput=False)
    mask48 = nc.declare_dram_parameter("mask48", [BLK, 1], f32, isOutput=False)
    elb48 = nc.declare_dram_parameter("elb48", [BLK, 1], f32, isOutput=False)
    out_logits = nc.declare_dram_parameter("out_logits", [NPC, 2], f32, isOutput=True)

    # ---- internal DRAM ----
    table = nc.dram_tensor("table", [NPAD, F_EXT], f16, addr_space="Shared")
    fe_local = nc.dram_tensor("fe_local", [NPC, F_EXT], f16)
    stats_in = nc.dram_tensor("stats_in", [HD, 2], f32)
    stats_out = nc.dram_tensor("stats_out", [HD, 2], f32, addr_space="Shared")

    with tile.TileContext(nc) as tc:
        with (
            tc.tile_pool(name="res", bufs=1) as res,
            tc.tile_pool(name="sb", bufs=2) as sb,
            tc.tile_pool(name="gat", bufs=2) as gat,
            tc.tile_pool(name="ps", bufs=4, space="PSUM") as ps,
        ):
            # resident tiles
            hv = res.tile([BLK, NB * HD], f16)       # v-major node features
            hT = res.tile([BLK, NB * HD], f16)       # ch-major
            rst = res.tile([BLK, NB * HD], f16)      # aggregation output
            scr = res.tile([BLK, NB * HD], f16)      # scratch strip
            fes = res.tile([BLK, NB * F_EXT], f16)   # [feat|el] staging
            er16 = res.tile([BLK, NB * 4], f16)      # er strip
            eidx = res.tile([BLK, CTOT], i32)
            snorm_sb = res.tile([BLK, NB], f32)
            snorm16 = res.tile([BLK, NB], f16)
            ident = res.tile([BLK, BLK], f16)
            ones_col = res.tile([BLK, 1], f16)
            Wsb = [res.tile([HD, HD + 8], f16, name=f"Wsb{i}") for i in range(3)]
            gam_sb = [res.tile([HD, 1], f32, name=f"gam{i}") for i in range(3)]
            bet_sb = [res.tile([HD, 1], f32, name=f"bet{i}") for i in range(3)]
            c1w = res.tile([HD, 64], f16)
            c1b = res.tile([64, 1], f32)
            c2wb = res.tile([65, 2], f16)
            emb_sb = res.tile([BLK, NB], i32)
            m48 = res.tile([BLK, 1], f32)
            eb48 = res.tile([BLK, 1], f32)
            lgs = res.tile([BLK, NB * 2], f32)       # logits staging

            make_identity(nc, ident[:])
            nc.vector.memset(ones_col[:], 1.0)
            nc.sync.dma_start(out=eidx[:], in_=edge_idx[:])
            nc.sync.dma_start(out=snorm_sb[:], in_=snorm[:])
            nc.sync.dma_start(out=emb_sb[:], in_=emb_idx[:])
            for i in range(3):
                nc.sync.dma_start(out=Wsb[i][:], in_=Wexts[i][:])
                nc.sync.dma_start(out=gam_sb[i][:], in_=gammas[i][:])
                nc.sync.dma_start(out=bet_sb[i][:], in_=betas[i][:])
            nc.sync.dma_start(out=c1w[:], in_=cls1_w[:])
            nc.sync.dma_start(out=c1b[:], in_=cls1_b[:])
            nc.sync.dma_start(out=c2wb[:], in_=cls2_wb[:])
            nc.sync.dma_start(out=m48[:], in_=mask48[:])
            nc.sync.dma_start(out=eb48[:], in_=elb48[:])
            nc.vector.tensor_copy(out=snorm16[:], in_=snorm_sb[:])

            # ---- embed gather: hv[p, b*128:(b+1)*128] = embed[x[node(b,p)]] ----
            for b in range(NB):
                nc.gpsimd.indirect_dma_start(
                    out=hv[:, b * HD:(b + 1) * HD], out_offset=None,
                    in_=embed[:],
                    in_offset=bass.IndirectOffsetOnAxis(
                        ap=emb_sb[:, b:b + 1], axis=0),
                )

            def elu_inplace(dst_ap, scr_ap):
                """dst = elu(dst) = max(x,0) + exp(min(x,0)) - 1 (fp16-safe)."""
                nc.vector.tensor_scalar_min(out=scr_ap, in0=dst_ap, scalar1=0.0)
                nc.scalar.activation(out=scr_ap, in_=scr_ap, func=AF.Exp)
                nc.vector.tensor_scalar(out=dst_ap, in0=dst_ap, scalar1=0.0,
                                        scalar2=-1.0, op0=OP.max, op1=OP.add)
                nc.vector.tensor_add(out=dst_ap, in0=dst_ap, in1=scr_ap)

            for li in range(3):
                residual = li > 0
                # ---- phase A: transpose hv -> hT, feat/el/er matmul ----
                # (for li>0, hT already holds this layer's input ch-major)
                if li == 0:
                    for b in range(NB):
                        pt = ps.tile([BLK, BLK], f16, space="PSUM", name="pt", tag="pt", bufs=2)
                        nc.tensor.transpose(out=pt[:],
                                            in_=hv[:, b * HD:(b + 1) * HD],
                                            identity=ident[:])
                        nc.vector.tensor_copy(out=hT[:, b * HD:(b + 1) * HD],
                                              in_=pt[:])
                for b in range(NB):
                    pf = ps.tile([BLK, HD + 8], f32, space="PSUM", name="pf", tag="pf", bufs=2)
                    nc.tensor.matmul(out=pf[:], lhsT=hT[:, b * HD:(b + 1) * HD],
                                     rhs=Wsb[li][:], start=True, stop=True)
                    nc.vector.tensor_copy(
                        out=fes[:, b * F_EXT:(b + 1) * F_EXT], in_=pf[:, 0:F_EXT])
                    nc.vector.tensor_copy(out=er16[:, b * 4:(b + 1) * 4],
                                          in_=pf[:, F_EXT:F_EXT + 4])
                # mask block-48 el: el*m48 + eb48 (-100 on dummy lanes)
                nc.vector.tensor_scalar(
                    out=fes[:, 48 * F_EXT + HD:49 * F_EXT],
                    in0=fes[:, 48 * F_EXT + HD:49 * F_EXT],
                    scalar1=m48[:], scalar2=eb48[:], op0=OP.mult, op1=OP.add)
                nc.sync.dma_start(
                    out=fe_local[:].rearrange("(b p) f -> p b f", p=BLK),
                    in_=fes[:].rearrange("p (b f) -> p b f", b=NB))
                nc.gpsimd.collective_compute(
                    "AllGather", OP.bypass, replica_groups=RG,
                    ins=[fe_local[:]], outs=[table[:]],
                )
                # ---- phase B: per-block gather + edge softmax + aggregation ----
                pst = ps.tile([HD, 2], f32, space="PSUM", name="pst",
                              tag="pst", bufs=1)
                for b in range(NB):
                    Cb = int(C[b])
                    ob = int(off[b])
                    gb = gat.tile([BLK, Cb, F_EXT], f16)
                    for c in range(Cb):
                        nc.gpsimd.indirect_dma_start(
                            out=gb[:, c, :], out_offset=None, in_=table[:],
                            in_offset=bass.IndirectOffsetOnAxis(
                                ap=eidx[:, ob + c:ob + c + 1], axis=0),
                        )
                    eb = sb.tile([BLK, Cb, H], f16)
                    er_ap = er16[:, b * 4:b * 4 + 4].unsqueeze(1).to_broadcast(
                        (BLK, Cb, H))
                    nc.vector.tensor_tensor(out=eb[:], in0=gb[:, :, HD:F_EXT],
                                            in1=er_ap, op=OP.add)
                    lb = sb.tile([BLK, Cb, H], f16)
                    nc.vector.tensor_scalar_mul(out=lb[:], in0=eb[:], scalar1=NEG)
                    nc.vector.tensor_tensor(out=eb[:], in0=eb[:], in1=lb[:],
                                            op=OP.max)
                    ee = sb.tile([BLK, Cb, H], f32)
                    nc.scalar.activation(out=ee[:], in_=eb[:], func=AF.Exp)
                    s4 = sb.tile([BLK, H], f32)
                    nc.vector.tensor_reduce(
                        out=s4[:], in_=ee[:].rearrange("p c h -> p h c"),
                        axis=mybir.AxisListType.X, op=OP.add)
                    nc.vector.tensor_scalar_add(out=s4[:], in0=s4[:],
                                                scalar1=1e-30)
                    rs = sb.tile([BLK, H], f32)
                    nc.vector.reciprocal(out=rs[:], in_=s4[:])
                    if li == 0:
                        nc.vector.tensor_scalar_mul(out=rs[:], in0=rs[:],
                                                    scalar1=snorm_sb[:, b:b + 1])
                    a16 = sb.tile([BLK, Cb, H], f16)
                    nc.vector.tensor_tensor(
                        out=a16[:], in0=ee[:],
                        in1=rs[:].unsqueeze(1).to_broadcast((BLK, Cb, H)),
                        op=OP.mult)
                    msg = gat.tile([BLK, Cb, H, D], f16)
                    nc.vector.tensor_tensor(
                        out=msg[:],
                        in0=gb[:, :, 0:HD].rearrange("p c (h d) -> p c h d", h=H),
                        in1=a16[:].unsqueeze(3).to_broadcast((BLK, Cb, H, D)),
                        op=OP.mult)
                    with nc.allow_low_precision(reason="msg sums fit fp16"):
                        nc.vector.tensor_reduce(
                            out=rst[:, b * HD:(b + 1) * HD].rearrange(
                                "p (h d) -> p h d", h=H),
                            in_=msg[:].rearrange("p c h d -> p h d c"),
                            axis=mybir.AxisListType.X, op=OP.add)
                    rb = rst[:, b * HD:(b + 1) * HD]
                    if residual:
                        nc.vector.tensor_add(out=rb, in0=rb,
                                             in1=hv[:, b * HD:(b + 1) * HD])
                        elu_inplace(rb, scr[:, b * HD:(b + 1) * HD])
                        nc.vector.tensor_scalar_mul(out=rb, in0=rb,
                                                    scalar1=snorm_sb[:, b:b + 1])
                    if b == NB - 1:
                        nc.vector.tensor_scalar_mul(out=rb, in0=rb,
                                                    scalar1=m48[:])
                    nc.scalar.activation(out=scr[:, b * HD:(b + 1) * HD],
                                         in_=rb, func=AF.Square)
                    nc.tensor.matmul(out=pst[:, 0:1], lhsT=rb,
                                     rhs=ones_col[:], start=(b == 0),
                                     stop=(b == NB - 1))
                    nc.tensor.matmul(out=pst[:, 1:2],
                                     lhsT=scr[:, b * HD:(b + 1) * HD],
                                     rhs=ones_col[:], start=(b == 0),
                                     stop=(b == NB - 1))
                # ---- phase C: BN coefficients ----
                st = sb.tile([HD, 2], f32)
                nc.vector.tensor_copy(out=st[:], in_=pst[:])
                nc.sync.dma_start(out=stats_in[:], in_=st[:])
                nc.gpsimd.collective_compute(
                    "AllReduce", OP.add, replica_groups=RG,
                    ins=[stats_in[:]], outs=[stats_out[:]],
                )
                st2 = sb.tile([HD, 2], f32)
                nc.sync.dma_start(out=st2[:], in_=stats_out[:])
                mu = sb.tile([HD, 1], f32)
                var = sb.tile([HD, 1], f32)
                sc = sb.tile([HD, 1], f32)
                bs = sb.tile([HD, 1], f32)
                nc.vector.tensor_scalar_mul(out=mu[:], in0=st2[:, 0:1],
                                            scalar1=1.0 / N)
                nc.vector.tensor_scalar_mul(out=var[:], in0=st2[:, 1:2],
                                            scalar1=1.0 / N)
                nc.vector.tensor_tensor(out=sc[:], in0=mu[:], in1=mu[:], op=OP.mult)
                nc.vector.tensor_tensor(out=var[:], in0=var[:], in1=sc[:],
                                        op=OP.subtract)
                nc.vector.tensor_scalar_add(out=var[:], in0=var[:], scalar1=BN_EPS)
                nc.scalar.activation(out=var[:], in_=var[:], func=AF.Sqrt)
                nc.vector.reciprocal(out=var[:], in_=var[:])
                nc.vector.tensor_tensor(out=sc[:], in0=gam_sb[li][:], in1=var[:],
                                        op=OP.mult)
                nc.vector.tensor_tensor(out=bs[:], in0=mu[:], in1=sc[:], op=OP.mult)
                nc.vector.tensor_tensor(out=bs[:], in0=bet_sb[li][:], in1=bs[:],
                                        op=OP.subtract)
                # transpose rst -> hT, BN apply, ELU
                for b in range(NB):
                    pt = ps.tile([BLK, BLK], f16, space="PSUM", name="pt", tag="pt", bufs=2)
                    nc.tensor.transpose(out=pt[:], in_=rst[:, b * HD:(b + 1) * HD],
                                        identity=ident[:])
                    nc.vector.tensor_copy(out=hT[:, b * HD:(b + 1) * HD], in_=pt[:])
                nc.scalar.activation(out=hT[:], in_=hT[:], func=AF.Identity,
                                     bias=bs[:], scale=sc[:])
                elu_inplace(hT[:], scr[:])
                if li < 2:
                    # back-transpose to v-major for next layer's residual
                    for b in range(NB):
                        pt = ps.tile([BLK, BLK], f16, space="PSUM", name="pt", tag="pt", bufs=2)
                        nc.tensor.transpose(out=pt[:],
                                            in_=hT[:, b * HD:(b + 1) * HD],
                                            identity=ident[:])
                        nc.vector.tensor_copy(out=hv[:, b * HD:(b + 1) * HD],
                                              in_=pt[:])

            # ---- classifier ----
            for b in range(NB):
                pz = ps.tile([BLK, 64], f32, space="PSUM", name="pz", tag="pz", bufs=1)
                nc.tensor.matmul(out=pz[:], lhsT=hT[:, b * HD:(b + 1) * HD],
                                 rhs=c1w[:], start=True, stop=True)
                zsb = sb.tile([BLK, 64], f16)
                nc.vector.tensor_copy(out=zsb[:], in_=pz[:])
                pzt = ps.tile([64, BLK], f16, space="PSUM", name="pzt", tag="pzt", bufs=1)
                nc.tensor.transpose(out=pzt[:], in_=zsb[:], identity=ident[:])
                zt = sb.tile([65, BLK], f16)
                nc.scalar.activation(out=zt[0:64, :], in_=pzt[:], func=AF.Relu,
                                     bias=c1b[:])
                nc.vector.memset(zt[64:65, :], 1.0)
                pl = ps.tile([BLK, 2], f32, space="PSUM", name="pl", tag="pl", bufs=1)
                nc.tensor.matmul(out=pl[:], lhsT=zt[:], rhs=c2wb[:],
                                 start=True, stop=True)
                nc.vector.tensor_copy(out=lgs[:, b * 2:(b + 1) * 2], in_=pl[:])
            nc.sync.dma_start(
                out=out_logits[:].rearrange("(b p) t -> p b t", p=BLK),
                in_=lgs[:].rearrange("p (b t) -> p b t", b=NB))
    nc.compile()
    return nc


def _prepare(inputs):
    """Build (nc, in_maps, plan) for the given full inputs."""
    inp = {k: np.asarray(v) for k, v in inputs.items()}
    plan = _build_plan(inp["src"], inp["dst"])
    C, off, CTOT = plan["C"], plan["off"], plan["CTOT"]
    edge_idx, nodes_kj = plan["edge_idx"], plan["nodes_kj"]

    key = CTOT
    if key not in _CACHE:
        _CACHE[key] = _build_program(C, off, CTOT)
    nc = _CACHE[key]

    # host-side weight prep
    def wext(W, al, ar):
        albd = np.zeros((HD, H), np.float32)
        arbd = np.zeros((HD, H), np.float32)
        for hh in range(H):
            albd[hh * D:(hh + 1) * D, hh] = al[hh]
            arbd[hh * D:(hh + 1) * D, hh] = ar[hh]
        return np.concatenate(
            [W, W @ albd, W @ arbd], axis=1).astype(np.float16)

    m48v = np.ones((BLK, 1), np.float32); m48v[106:] = 0.0
    e48v = np.zeros((BLK, 1), np.float32); e48v[106:] = EL_DUMMY
    common = {
        "mask48": m48v, "elb48": e48v,
        "embed": inp["embed"].astype(np.float16),
        "cls1_w": inp["cls1_w"].astype(np.float16),
        "cls1_b": inp["cls1_b"].reshape(64, 1).astype(np.float32),
        "cls2_wb": np.concatenate(
            [inp["cls2_w"], inp["cls2_b"].reshape(1, 2)], axis=0
        ).astype(np.float16),
    }
    for i in range(3):
        common[f"Wext{i}"] = wext(inp[f"W{i}"], inp[f"al{i}"], inp[f"ar{i}"])
        common[f"gamma{i}"] = inp[f"gamma{i}"].reshape(HD, 1).astype(np.float32)
        common[f"beta{i}"] = inp[f"beta{i}"].reshape(HD, 1).astype(np.float32)

    x = inp["x"].astype(np.int64)
    sn = inp["snorm_n"].reshape(-1).astype(np.float32)
    in_maps = []
    for k in range(NCORES):
        nk = nodes_kj[k]                      # [NPC] node ids, -1 dummy
        m = nk >= 0
        ei = np.zeros(NPC, dtype=np.int32)
        ei[m] = x[nk[m]].astype(np.int32)
        snk = np.zeros(NPC, dtype=np.float32)
        snk[m] = sn[nk[m]]
        im = dict(common)
        im["emb_idx"] = ei.reshape(NB, BLK).T.copy()      # [128, NB]
        im["snorm"] = snk.reshape(NB, BLK).T.copy()
        im["edge_idx"] = plan["edge_idx"][k]
        in_maps.append(im)

    return nc, in_maps, plan


def _unshard(res, plan):
    nodes_kj = plan["nodes_kj"]
    out = np.zeros((N, 2), np.float32)
    for k in range(NCORES):
        lg = res[k]["out_logits"]             # [NPC, 2]
        nk = nodes_kj[k]
        m = nk >= 0
        out[nk[m]] = lg[m]
    return out


def kernel(**inputs):
    from concourse.bass_utils import run_bass_kernel_spmd

    nc, in_maps, plan = _prepare(inputs)
    res = run_bass_kernel_spmd(nc, in_maps, list(range(NCORES))).results
    return _unshard(res, plan)
